# revision 34
# baseline (speedup 1.0000x reference)
"""EnergyGCN Trainium2 kernel: 8-core SPMD Bass/Tile implementation (v2).

Strategy (node sharding):
  - 50000 nodes sharded contiguously across 8 cores (6250 dest nodes/core,
    padded to 6272 = 49*128 table rows per core).
  - Per layer: each core computes hr = h @ Wr[l].T for its shard, scales rows
    by dinv (h~ = dinv * hr, bf16), AllGathers h~ into a full [50176,128] bf16
    HBM table.
  - Edges are dest-sorted on the host, grouped into 128-edge chunks per
    128-dest block.  Messages h~[col] are fetched with ONE batched indirect
    DMA per dest block (int32 row indices, [128, nchunks] offset AP) and
    aggregated on the TensorEngine with one-hot segment matrices S generated
    on-chip by DVE is_equal in bf16; PSUM accumulates
        psum[d,:] = sum_e 1[dest_e==d] * h~[col_e]        (chunks, bf16 mms)
                  + (-2*deg_d) * h~[d]                    (identity inject 1)
                  + gate_l * (-1/(3*dinv_d)) * h0[d]      (identity inject 2)
    and the epilogue h0_new = (-3*dinv_d) * psum gives exactly
        h0_new = gate_l*h0 + 6*hr - 3*ahat(hr).
  - A persistent bf16 relu(h0) tile per block is refreshed in the epilogue so
    the next layer's lin_right phase has no extra relu pass.

Reference math:
    h = relu(x @ W1 + b1); h0 = h
    for l: hr = h @ Wr[l].T ; hn = 6hr - 3*ahat(hr)
           h0 = (1+tanh(eps[l]))*h0 + hn ; h = relu(h0)
    out = h @ W2 + b2
with ahat(y) = segment_sum(w[:,None]*y[col], row), w = dinv[row]*dinv[col],
self-loops appended, deg = counts of row, dinv = rsqrt(deg).
"""

import math

import numpy as np

try:
    import ml_dtypes

    BF16_NP = ml_dtypes.bfloat16
except ImportError:  # pragma: no cover
    BF16_NP = None

import concourse.bacc as bacc
import concourse.bass as bass
import concourse.mybir as mybir
import concourse.tile as tile
from concourse import bass_utils

F32 = mybir.dt.float32
BF16 = mybir.dt.bfloat16
AF = mybir.ActivationFunctionType
ALU = mybir.AluOpType

N_NODES = 50000
D_IN, D_H, D_OUT = 256, 128, 64
N_LAYERS = 4
N_CORES = 8


class Cfg:
    def __init__(self, n=N_NODES, n_cores=N_CORES, d_in=D_IN, d_h=D_H,
                 d_out=D_OUT, n_layers=N_LAYERS, sgen_batch=8, gather_bufs=2,
                 sg_blocks=3, dma_scratch=16384, use_dma_gather=True,
                 call_chunks=8):
        self.n = n
        self.n_cores = n_cores
        self.d_in = d_in
        self.d_h = d_h
        self.d_out = d_out
        self.n_layers = n_layers
        self.sgen_batch = sgen_batch
        self.gather_bufs = gather_bufs
        self.sg_blocks = sg_blocks
        self.dma_scratch = dma_scratch
        self.use_dma_gather = use_dma_gather
        self.call_chunks = call_chunks
        assert n % n_cores == 0
        self.nsh = n // n_cores              # dest nodes per core
        self.nb = (self.nsh + 127) // 128    # dest blocks per core
        self.nshp = self.nb * 128            # padded shard rows
        self.ntp = self.nshp * n_cores       # padded table rows
        # int16 gather indices: split the table into two row windows
        self.half_rows = (n_cores // 2) * self.nshp
        assert self.half_rows < 32768
        assert self.ntp - self.half_rows < 32768


class Plan:
    pass


def preprocess(edge_index: np.ndarray, cfg: Cfg) -> Plan:
    n, P, nsh, nb = cfg.n, cfg.n_cores, cfg.nsh, cfg.nb
    row = np.concatenate([edge_index[0], np.arange(n, dtype=np.int64)])
    col = np.concatenate([edge_index[1], np.arange(n, dtype=np.int64)])
    deg = np.bincount(row, minlength=n).astype(np.float64)
    dinv = np.where(deg > 0, 1.0 / np.sqrt(deg), 0.0)
    # padded table row for each source node
    tblrow = (col // nsh) * cfg.nshp + (col % nsh)

    core = row // nsh
    per_core_edges = []
    counts = np.zeros((P, nb, 2), dtype=np.int64)
    for r in range(P):
        m = core == r
        rr = (row[m] - r * nsh).astype(np.int64)
        tr = tblrow[m].astype(np.int64)
        g = (tr >= cfg.half_rows).astype(np.int64)
        blk = rr // 128
        per_core_edges.append((rr, tr, g, blk))
        np.add.at(counts[r], (blk, g), 1)

    nchunks = (counts.max(axis=0) + 127) // 128            # [nb, 2]

    # chunk layout: supergroups of sg_blocks blocks; within each supergroup,
    # group-0 chunks for its blocks in order, then group-1 chunks.  One
    # dma_gather per (sg, g).
    chunk_off = {}
    sg_entries = []
    off = 0
    n_sg = (nb + cfg.sg_blocks - 1) // cfg.sg_blocks
    for s in range(n_sg):
        blocks = list(range(s * cfg.sg_blocks,
                            min((s + 1) * cfg.sg_blocks, nb)))
        entries = []
        for g in (0, 1):
            c0 = off
            for b in blocks:
                chunk_off[(b, g)] = off
                off += int(nchunks[b, g])
            entries.append((g, c0, off - c0))
        sg_entries.append((blocks, entries))
    total_chunks = off
    total_slots = total_chunks * 128

    per_core = []
    for r in range(P):
        rr, tr, g, blk = per_core_edges[r]
        idx = np.zeros(total_slots, dtype=np.int64)
        dstrel = np.full(total_slots, -1.0, dtype=np.float64)
        for b in range(nb):
            for gg in (0, 1):
                m = (blk == b) & (g == gg)
                k = int(m.sum())
                if k == 0:
                    continue
                o = chunk_off[(b, gg)] * 128
                idx[o:o + k] = tr[m] - gg * cfg.half_rows
                dstrel[o:o + k] = (rr[m] - b * 128).astype(np.float64)
        # int16 indices wrapped over 16 partitions (idx i -> [i%16, i//16]),
        # replicated across the 8 groups of 16 partitions for the Q7 cores
        idxw = np.tile(idx.astype(np.int16).reshape(-1, 16).T, (8, 1))
        dstrel128 = np.ascontiguousarray(
            dstrel.reshape(total_chunks, 128).T.astype(BF16_NP))

        dloc = dinv[r * nsh:(r + 1) * nsh]
        degloc = deg[r * nsh:(r + 1) * nsh]

        def colmat(v):
            out = np.zeros((nb * 128,), dtype=np.float64)
            out[:nsh] = v
            return np.ascontiguousarray(out.reshape(nb, 128).T.astype(np.float32))

        gvec = np.zeros(total_slots, dtype=np.int64)
        for b in range(nb):
            for gg in (0, 1):
                o = chunk_off[(b, gg)] * 128
                gvec[o:o + int(nchunks[b, gg]) * 128] = gg
        idx_abs = idx + gvec * cfg.half_rows
        per_core.append(dict(
            idx_tbl=np.ascontiguousarray(idxw),
            idx_raw=idx.copy(),
            idx128=np.ascontiguousarray(
                idx_abs.astype(np.int32).reshape(total_chunks, 128).T),
            dstrel=dstrel128,
            dinv_cols=colmat(dloc),
            s_cols=colmat(-2.0 * degloc),
            s2_cols=colmat(np.where(dloc > 0, -1.0 / (3.0 * dloc), 0.0)),
            m3dinv_cols=colmat(-3.0 * dloc),
        ))

    plan = Plan()
    plan.cfg = cfg
    plan.nchunks = nchunks
    plan.chunk_off = chunk_off
    plan.sg_entries = sg_entries
    plan.total_chunks = total_chunks
    plan.per_core = per_core
    return plan


def build_bass(plan: Plan, gates, debug_dump=False):
    cfg = plan.cfg
    nsh, nb, P = cfg.nsh, cfg.nb, cfg.n_cores
    H, DI, DO, L = cfg.d_h, cfg.d_in, cfg.d_out, cfg.n_layers
    total_chunks = plan.total_chunks
    maxcall = max((nch for (_, entries) in plan.sg_entries
                   for (_, _, nch) in entries), default=1)
    SB = cfg.sgen_batch

    nc = bacc.Bacc("TRN2", target_bir_lowering=False, debug=False,
                   num_devices=P,
                   dynamic_dma_scratch_size=cfg.dma_scratch)

    KI = DI // 128
    xT = nc.dram_tensor("xT", [128, KI * nsh], BF16, kind="ExternalInput")
    W1 = nc.dram_tensor("W1", [128, KI * H], BF16, kind="ExternalInput")
    b1r = nc.dram_tensor("b1r", [1, H], BF16, kind="ExternalInput")
    WrT = nc.dram_tensor("WrT", [128, L * H], BF16, kind="ExternalInput")
    W2 = nc.dram_tensor("W2", [H, DO], BF16, kind="ExternalInput")
    b2r = nc.dram_tensor("b2r", [1, DO], BF16, kind="ExternalInput")
    idx_tbl_d = nc.dram_tensor("idx_tbl", [128, total_chunks * 8],
                               mybir.dt.int16, kind="ExternalInput")
    if not cfg.use_dma_gather:
        idx32_d = nc.dram_tensor("idx32_tbl", [128, total_chunks],
                                 mybir.dt.int32, kind="ExternalInput")
    iota_d = nc.dram_tensor("iota_in", [128, SB * 128], BF16,
                            kind="ExternalInput")
    ident_d = nc.dram_tensor("ident_in", [128, 128], BF16,
                             kind="ExternalInput")
    dstrel_d = nc.dram_tensor("dstrel", [128, total_chunks], BF16,
                              kind="ExternalInput")
    dinv_d = nc.dram_tensor("dinv_cols", [128, nb], F32, kind="ExternalInput")
    s_d = nc.dram_tensor("s_cols", [128, nb], F32, kind="ExternalInput")
    s2_d = nc.dram_tensor("s2_cols", [128, nb], F32, kind="ExternalInput")
    m3_d = nc.dram_tensor("m3dinv_cols", [128, nb], F32, kind="ExternalInput")
    out_d = nc.dram_tensor("out", [nsh, DO], F32, kind="ExternalOutput")
    if debug_dump:
        dbg_h0a = nc.dram_tensor("dbg_h0a", [cfg.nshp, 128], F32,
                                 kind="ExternalOutput")
        dbg_htl = nc.dram_tensor("dbg_htl", [128, nb * H], F32,
                                 kind="ExternalOutput")
        dbg_tbl = nc.dram_tensor("dbg_tbl", [cfg.ntp, H], BF16,
                                 kind="ExternalOutput")
        dbg_mb = nc.dram_tensor("dbg_mb", [128, maxcall, H],
                                BF16, kind="ExternalOutput")
        dbg_h0b = nc.dram_tensor("dbg_h0b", [cfg.nshp, 128], F32,
                                 kind="ExternalOutput")

    last_rows = nsh - (nb - 1) * 128

    with tile.TileContext(nc) as tc:
        with (
            tc.tile_pool(name="const", bufs=1) as cpool,
            tc.tile_pool(name="io", bufs=1) as iopool,
            tc.tile_pool(name="work", bufs=3) as work,
            tc.tile_pool(name="sgen", bufs=3) as sgen_pool,
            tc.tile_pool(name="gbuf", bufs=cfg.gather_bufs) as gpool,
            tc.tile_pool(name="pt", bufs=2, space="PSUM") as pt_pool,
            tc.tile_pool(name="ph", bufs=2, space="PSUM") as ph_pool,
            tc.tile_pool(name="pagg", bufs=2, space="PSUM") as pagg_pool,
            tc.tile_pool(name="dram", bufs=2, space="DRAM") as dram,
        ):
            # persistent per-block state
            h0_t = [cpool.tile([128, 128], F32, tag=f"h0_{b}", name=f"h0_{b}")
                    for b in range(nb)]
            hrel_t = [cpool.tile([128, 128], BF16, tag=f"hr_{b}",
                                 name=f"hr_{b}") for b in range(nb)]
            htl_all = cpool.tile([128, nb * H], BF16, name="htl_all")
            for b in range(nb):
                nc.vector.memset(h0_t[b][:], 0.0)
                nc.vector.memset(hrel_t[b][:], 0.0)
            nc.vector.memset(htl_all[:], 0.0)

            idx_sb = cpool.tile([128, total_chunks * 8], mybir.dt.int16)
            nc.sync.dma_start(idx_sb[:], idx_tbl_d[:, :])
            if not cfg.use_dma_gather:
                idx32_sb = cpool.tile([128, total_chunks], mybir.dt.int32)
                nc.sync.dma_start(idx32_sb[:], idx32_d[:, :])
            dstrel_sb = cpool.tile([128, total_chunks], BF16)
            nc.sync.dma_start(dstrel_sb[:], dstrel_d[:, :])
            dinv_sb = cpool.tile([128, nb], F32)
            nc.sync.dma_start(dinv_sb[:], dinv_d[:, :])
            s_sb = cpool.tile([128, nb], F32)
            nc.sync.dma_start(s_sb[:], s_d[:, :])
            s2_sb = cpool.tile([128, nb], F32)
            nc.sync.dma_start(s2_sb[:], s2_d[:, :])
            m3_sb = cpool.tile([128, nb], F32)
            nc.sync.dma_start(m3_sb[:], m3_d[:, :])

            W1_sb = cpool.tile([128, KI * H], BF16)
            nc.sync.dma_start(W1_sb[:], W1[:, :])
            b1_sb = cpool.tile([1, H], BF16)
            nc.sync.dma_start(b1_sb[:], b1r[:, :])
            WrT_sb = cpool.tile([128, L * H], BF16)
            nc.sync.dma_start(WrT_sb[:], WrT[:, :])
            W2_sb = cpool.tile([H, DO], BF16)
            nc.sync.dma_start(W2_sb[:], W2[:, :])
            b2_sb = cpool.tile([1, DO], BF16)
            nc.sync.dma_start(b2_sb[:], b2r[:, :])
            ones_sb = cpool.tile([1, 128], BF16)
            nc.vector.memset(ones_sb[:], 1.0)

            iota_sb = cpool.tile([128, SB * 128], BF16)
            nc.sync.dma_start(iota_sb[:], iota_d[:, :])
            ident = cpool.tile([128, 128], BF16)
            nc.sync.dma_start(ident[:], ident_d[:, :])
            Is_t = [cpool.tile([128, 128], BF16, tag=f"is_{b}",
                               name=f"is_{b}") for b in range(nb)]
            for b in range(nb):
                nc.vector.tensor_scalar(Is_t[b][:], ident[:],
                                        s_sb[:, b:b + 1], None, op0=ALU.mult)

            # ---- lin1: h0 = relu(x @ W1 + b1) ----
            with tc.tile_pool(name="xio", bufs=1) as xpool:
                xs_all = xpool.tile([128, KI * nsh], BF16, name="xs_all")
                nc.sync.dma_start(xs_all[:], xT[:, :])
                for b in range(nb):
                    rows = last_rows if b == nb - 1 else 128
                    ps = ph_pool.tile([128, H], F32, tag="ph")
                    for k in range(KI):
                        nc.tensor.matmul(
                            ps[:rows, :],
                            xs_all[:, k * nsh + b * 128:
                                   k * nsh + b * 128 + rows],
                            W1_sb[:, k * H:(k + 1) * H],
                            start=(k == 0), stop=False)
                    nc.tensor.matmul(ps[:rows, :], ones_sb[:, :rows],
                                     b1_sb[:], start=False, stop=True)
                    nc.scalar.activation(h0_t[b][:rows, :], ps[:rows, :],
                                         AF.Relu)
                    nc.scalar.activation(hrel_t[b][:rows, :], ps[:rows, :],
                                         AF.Relu)

            if debug_dump:
                for b in range(nb):
                    d0 = work.tile([128, 128], F32, tag="dbgc")
                    nc.vector.tensor_copy(d0[:], h0_t[b][:])
                    nc.sync.dma_start(
                        dbg_h0a[b * 128:(b + 1) * 128, :], d0[:])

            # ---- layers ----
            for l in range(L):
                bounce = dram.tile([cfg.nshp, H], BF16, tag="bounce")
                table = dram.tile([cfg.ntp, H], BF16, tag="table",
                                  addr_space="Shared")
                # lin_right + dinv scale into the bounce staging tile
                for b in range(nb):
                    rows = last_rows if b == nb - 1 else 128
                    ptp = pt_pool.tile([128, 128], BF16, tag="pt")
                    nc.tensor.transpose(ptp[:], hrel_t[b][:], ident[:])
                    hT = work.tile([128, 128], BF16, tag="hT")
                    nc.vector.tensor_copy(hT[:], ptp[:])
                    php = ph_pool.tile([128, H], F32, tag="ph")
                    nc.tensor.matmul(php[:rows, :], hT[:, :rows],
                                     WrT_sb[:, l * H:(l + 1) * H],
                                     start=True, stop=True)
                    nc.scalar.activation(htl_all[:rows, b * H:(b + 1) * H],
                                         php[:rows, :], AF.Copy,
                                         scale=dinv_sb[:rows, b:b + 1])
                nc.sync.dma_start(
                    bounce[:].rearrange("(b p) h -> p b h", p=128),
                    htl_all[:].rearrange("p (b h) -> p b h", h=H))
                if debug_dump and l == 0:
                    dh = work.tile([128, nb * H], F32, tag="dbghtl")
                    nc.vector.tensor_copy(dh[:], htl_all[:])
                    nc.sync.dma_start(dbg_htl[:, :], dh[:])
                nc.gpsimd.collective_compute(
                    "AllGather", ALU.bypass,
                    replica_groups=[list(range(P))],
                    ins=[bounce.opt()], outs=[table.opt()],
                )

                tbl = table.opt()
                if debug_dump and l == 0:
                    nc.sync.dma_start(dbg_tbl[:, :], tbl)
                for (blocks, entries) in plan.sg_entries:
                    bufs = {}
                    call_c0 = {}
                    for (g, c0, nch) in entries:
                        if nch == 0:
                            continue
                        mbg = gpool.tile([128, maxcall, H], BF16, tag=f"mb{g}")
                        if cfg.use_dma_gather:
                            src = tbl[g * cfg.half_rows:
                                      g * cfg.half_rows + cfg.half_rows, :]
                            # HW SWDGE ring limit: <=1024 descriptors/call
                            W = cfg.call_chunks
                            for w0 in range(0, nch, W):
                                wn = min(W, nch - w0)
                                nc.gpsimd.dma_gather(
                                    mbg[:, w0:w0 + wn, :], src,
                                    idx_sb[:, (c0 + w0) * 8:
                                           (c0 + w0 + wn) * 8],
                                    wn * 128, wn * 128, H)
                        else:
                            for q in range(nch):
                                nc.gpsimd.indirect_dma_start(
                                    mbg[:, q, :], None, tbl,
                                    bass.IndirectOffsetOnAxis(
                                        ap=idx32_sb[:, c0 + q:c0 + q + 1],
                                        axis=0))
                        bufs[g] = mbg
                        call_c0[g] = c0
                        if debug_dump and l == 0 and blocks[0] == 0 and g == 0:
                            nc.sync.dma_start(dbg_mb[:, :nch, :],
                                              mbg[:, :nch, :])
                    for b in blocks:
                        rows = last_rows if b == nb - 1 else 128
                        nch_tot = int(plan.nchunks[b, 0] + plan.nchunks[b, 1])
                        pa = pagg_pool.tile([128, H], F32, tag="pagg")
                        nc.tensor.matmul(pa[:], Is_t[b][:],
                                         htl_all[:, b * H:(b + 1) * H],
                                         start=True, stop=False)
                        i2 = work.tile([128, 128], F32, tag="i2")
                        nc.vector.tensor_scalar(i2[:], ident[:],
                                                s2_sb[:, b:b + 1], gates[l],
                                                op0=ALU.mult, op1=ALU.mult)
                        nc.tensor.matmul(pa[:], i2[:], h0_t[b][:],
                                         start=False, stop=(nch_tot == 0))
                        done = 0
                        for g in (0, 1):
                            nch_bg = int(plan.nchunks[b, g])
                            if nch_bg == 0:
                                continue
                            c0 = plan.chunk_off[(b, g)]
                            loc = c0 - call_c0[g]
                            mbg = bufs[g]
                            for t0 in range(0, nch_bg, SB):
                                tn = min(SB, nch_bg - t0)
                                sg_t = sgen_pool.tile([128, SB * 128], BF16,
                                                      tag="sg")
                                dsl = dstrel_sb[:, c0 + t0:c0 + t0 + tn]
                                nc.vector.tensor_tensor(
                                    sg_t[:, :tn * 128].rearrange(
                                        "p (c d) -> p c d", d=128),
                                    dsl.unsqueeze(2).broadcast_to(
                                        [128, tn, 128]),
                                    iota_sb[:, :tn * 128].rearrange(
                                        "p (c d) -> p c d", d=128),
                                    op=ALU.is_equal)
                                for t in range(tn):
                                    done += 1
                                    nc.tensor.matmul(
                                        pa[:],
                                        sg_t[:, t * 128:(t + 1) * 128],
                                        mbg[:, loc + t0 + t, :],
                                        start=False, stop=(done == nch_tot))
                        nc.scalar.activation(h0_t[b][:rows, :], pa[:rows, :],
                                             AF.Copy,
                                             scale=m3_sb[:rows, b:b + 1])
                        nc.scalar.activation(hrel_t[b][:rows, :],
                                             pa[:rows, :], AF.Relu,
                                             scale=m3_sb[:rows, b:b + 1])

                if debug_dump and l == 0:
                    for b in range(nb):
                        d1 = work.tile([128, 128], F32, tag="dbgc")
                        nc.vector.tensor_copy(d1[:], h0_t[b][:])
                        nc.sync.dma_start(
                            dbg_h0b[b * 128:(b + 1) * 128, :], d1[:])

            # ---- lin2 ----
            out_all = iopool.tile([128, nb * DO], F32, name="out_all")
            for b in range(nb):
                rows = last_rows if b == nb - 1 else 128
                ptp = pt_pool.tile([128, 128], BF16, tag="pt")
                nc.tensor.transpose(ptp[:], hrel_t[b][:], ident[:])
                hT = work.tile([128, 128], BF16, tag="hT")
                nc.vector.tensor_copy(hT[:], ptp[:])
                po = ph_pool.tile([128, DO], F32, tag="po")
                nc.tensor.matmul(po[:rows, :], hT[:, :rows], W2_sb[:, :],
                                 start=True, stop=False)
                nc.tensor.matmul(po[:rows, :], ones_sb[:, :rows], b2_sb[:],
                                 start=False, stop=True)
                nc.scalar.activation(out_all[:rows, b * DO:(b + 1) * DO],
                                     po[:rows, :], AF.Copy)
            nfull = (nb - 1) * 128
            nc.sync.dma_start(
                out_d[:nfull, :].rearrange("(b p) o -> p b o", p=128),
                out_all[:, :(nb - 1) * DO].rearrange("p (b o) -> p b o", o=DO))
            nc.sync.dma_start(
                out_d[nfull:nsh, :],
                out_all[:last_rows, (nb - 1) * DO:nb * DO])

    nc.finalize()
    return nc


def make_in_maps(plan: Plan, x, W1, b1, Wr, W2, b2):
    cfg = plan.cfg
    nsh = cfg.nsh
    KI = cfg.d_in // 128
    W1m = np.ascontiguousarray(
        np.asarray(W1, np.float32).reshape(KI, 128, cfg.d_h)
        .transpose(1, 0, 2).reshape(128, KI * cfg.d_h)).astype(BF16_NP)
    # WrT[k, l*H+j] = Wr[l, j, k]
    WrTm = np.ascontiguousarray(
        np.asarray(Wr, np.float32).transpose(2, 0, 1).reshape(128, -1)
    ).astype(BF16_NP)
    iota_in = np.ascontiguousarray(
        np.tile(np.arange(128, dtype=np.float32),
                cfg.sgen_batch)[None, :].repeat(128, 0)).astype(BF16_NP)
    ident_in = np.eye(128, dtype=np.float32).astype(BF16_NP)
    common = dict(
        W1=W1m,
        b1r=np.asarray(b1, np.float32).reshape(1, -1).astype(BF16_NP),
        WrT=WrTm,
        W2=np.ascontiguousarray(np.asarray(W2, np.float32)).astype(BF16_NP),
        b2r=np.asarray(b2, np.float32).reshape(1, -1).astype(BF16_NP),
        iota_in=iota_in, ident_in=ident_in,
    )
    x = np.asarray(x, np.float32)
    in_maps = []
    for r in range(cfg.n_cores):
        pc = plan.per_core[r]
        # xT[p, k*nsh + c] = x[c, k*128 + p]
        xTm = np.ascontiguousarray(
            x[r * nsh:(r + 1) * nsh].T
            .reshape(KI, 128, nsh).transpose(1, 0, 2).reshape(128, KI * nsh)
        ).astype(BF16_NP)
        m = dict(common)
        m.update(
            xT=xTm, idx_tbl=pc["idx_tbl"], dstrel=pc["dstrel"],
            dinv_cols=pc["dinv_cols"], s_cols=pc["s_cols"],
            s2_cols=pc["s2_cols"], m3dinv_cols=pc["m3dinv_cols"],
        )
        if not cfg.use_dma_gather:
            m["idx32_tbl"] = pc["idx128"]
        in_maps.append(m)
    return in_maps


_cache = {}


def kernel(x, W1, b1, Wr, eps, W2, b2, edge_index, *, trace=False, cfg=None,
           debug_dump=False):
    cfg = cfg or Cfg()
    x = np.asarray(x)
    edge_index = np.asarray(edge_index)
    gates = [float(1.0 + math.tanh(float(e))) for e in np.asarray(eps)]

    ck = hash((edge_index.tobytes(), tuple(gates), cfg.n, cfg.n_cores,
               debug_dump))
    if ck not in _cache:
        plan = preprocess(edge_index, cfg)
        nc = build_bass(plan, gates, debug_dump=debug_dump)
        _cache.clear()
        _cache[ck] = (plan, nc)
    plan, nc = _cache[ck]
    kernel.last_plan = plan

    in_maps = make_in_maps(plan, x, W1, b1, Wr, W2, b2)
    try:
        res = bass_utils.run_bass_kernel_spmd(
            nc, in_maps, core_ids=list(range(cfg.n_cores)), trace=trace)
    except ModuleNotFoundError:
        # axon NTFF profiling hook unavailable in this container
        res = bass_utils.run_bass_kernel_spmd(
            nc, in_maps, core_ids=list(range(cfg.n_cores)), trace=False)
    out = np.concatenate([r["out"] for r in res.results], axis=0)
    kernel.last_results = res
    return out.astype(np.float32)
